# revision 1
# baseline (speedup 1.0000x reference)
"""ClassWeightedModalDownSampler Trainium2 kernel.

Problem: labels [4, 1024, 2048] int (values 0..19), class_weights [20] f32,
dsf=8.  Output modes [4, 128, 256]: per non-overlapping 8x8 patch, the
argmax over classes of (class histogram * class_weights), first-index
tie-break (jnp.argmax semantics).

Strategy (pure data parallel over 8 cores, each core takes 64 patch rows
= 512 label rows):
  host:  cast labels to bf16 (exact for 0..19) and pre-transpose each
         core's shard to X[p, n] with p = w mod 128 and, per wc-half,
         n = r*512 + wcl*64 + prow (r = row mod 8, prow = row div 8,
         wcl = (w div 128) mod 8), so every matmul rhs is one contiguous
         512-column slice.  Upload is 2 MiB/core.
  compares (one-hot planes, [128, 4096] per wc-half): split across
         DVE (bf16 out at 4x -> normal matmuls; fp8 out at 2x ->
         DoubleRow matmuls), ACT (relu(1-(x-c)^2), exact 0/1 after fp8
         cast), and GPSIMD, balancing engine busy time.
  PE:    per class, PSUM-accumulated matmuls with a block-diagonal ones
         lhsT contract the 8 q-pixels (partition groups of 8) and the 8
         patch rows (8 accumulating matmuls, or 4 fp8 DoubleRow ones)
         -> PSUM [128, 512] banks, 8 classes each: partition 16*j + jj,
         free wcl*64 + prow.
  ACT:   encode E = count * (64*w_c) - c via Identity with per-partition
         scale/bias APs.  Exact for integer class_weights (the contract);
         for fractional weights classes whose fl(w*count) differ by less
         than 19/64 could misrank vs the fp32 reference.
  DVE:   max-fold across banks and partition groups (partition moves via
         SBUF->SBUF DMA), 2-op decode (casts fused), emit int32.
  host:  gather per-core outputs and rearrange to [4, 128, 256].
"""

import numpy as np
import ml_dtypes

import concourse.bass as bass
import concourse.mybir as mybir
import concourse.tile as tile
from concourse import bacc
from concourse.bass_utils import run_bass_kernel_spmd

NCORES = 8
B, H, W = 4, 1024, 2048
DSF = 8
NCLS = 20
GH, GW = H // DSF, W // DSF  # 128, 256 output grid
ROWS = (B * H) // NCORES     # 512 label rows per core
PROWS = ROWS // DSF          # 64 patch rows per core
P = 128
WC = W // P                  # 16 column chunks of 128
HALVES = 2
WCH = WC // HALVES           # 8 chunks per half
FREE = WC * ROWS             # 8192
HFREE = FREE // HALVES       # 4096
NBANK = 3                    # psum banks per half (classes 0-7, 8-15, 16-19x2)

_DT = mybir.dt

# per-class compare engine: spread across DVE (bf16->normal matmul and
# fp8->DoubleRow), ACT (square/relu trick), GPSIMD, to balance busy time.
DVE_BF16 = (0, 1, 2, 3, 8, 9, 10, 11, 16, 17)
DVE_FP8 = (4, 12, 14, 18, 19)
ACT_CLS = (5, 6, 13)
GP_CLS = (7, 15)

# Set by test.py to request a traced run.
TRACE = False
LAST_RESULTS = None


def _aux_arrays(class_weights: np.ndarray):
    """Host-built constant inputs: lhsT matrices and encode scalars."""
    # lhsT blocks: 12 matrices [128, 128] bf16, flattened to [128, 12*128].
    # j = 0..7: ones at (p, 16*j + p//8).   (classes c with c%8 == j)
    # j = 8..11 (jd = j-8): doubled block for classes 16..19:
    #   ones at (p, 16*jd + p//8) and (p, 64 + 16*jd + p//8).
    lhst = np.zeros((P, 12 * P), dtype=np.float32)
    for j in range(8):
        for p in range(P):
            lhst[p, j * P + 16 * j + p // 8] = 1.0
    for jd in range(4):
        for p in range(P):
            base = (8 + jd) * P
            lhst[p, base + 16 * jd + p // 8] = 1.0
            lhst[p, base + 64 + 16 * jd + p // 8] = 1.0
    lhst_bf = lhst.astype(ml_dtypes.bfloat16)
    # DoubleRow wants lhsT [K, 2, M]: two consecutive [P, 128] copies of
    # each block (k-tile pair as contiguous free blocks).
    l8 = lhst.reshape(P, 12, 1, P)
    l8 = np.concatenate([l8, l8], axis=2).reshape(P, 12 * 2 * P)
    lhst_f8 = l8.astype(ml_dtypes.float8_e4m3)

    # ACT compare biases: Square pass bias -c, Relu pass bias 1.0
    actb = np.zeros((P, 2 * len(ACT_CLS)), dtype=np.float32)
    for i, c in enumerate(ACT_CLS):
        actb[:, 2 * i] = -float(c)
        actb[:, 2 * i + 1] = 1.0

    # encode scalars per psum bank g (E = 64*w_c*count - c):
    # partition p holds class
    #   g<2:  c = 8*g + p//16
    #   g==2: c = 16 + (p//16) % 4
    wscale = np.zeros((P, NBANK), dtype=np.float32)
    wbias = np.zeros((P, NBANK), dtype=np.float32)
    w = np.asarray(class_weights, dtype=np.float32)
    for g in range(NBANK):
        for p in range(P):
            c = 8 * g + p // 16 if g < 2 else 16 + (p // 16) % 4
            wscale[p, g] = 64.0 * w[c]
            wbias[p, g] = float(-c)
    return lhst_bf, lhst_f8, actb, wscale, wbias


def _build():
    """Build the SPMD Bass kernel (same NEFF on all 8 cores)."""
    nc = bacc.Bacc(
        "TRN2",
        target_bir_lowering=False,
        debug=False,
        num_devices=NCORES,
    )
    x_d = nc.dram_tensor("x", [P, FREE], _DT.bfloat16, kind="ExternalInput").ap()
    lhstb_d = nc.dram_tensor("lhstb", [P, 12 * P], _DT.bfloat16, kind="ExternalInput").ap()
    lhst8_d = nc.dram_tensor("lhst8", [P, 2 * 12 * P], _DT.float8e4, kind="ExternalInput").ap()
    actb_d = nc.dram_tensor("actb", [P, 2 * len(ACT_CLS)], _DT.float32, kind="ExternalInput").ap()
    wscale_d = nc.dram_tensor("wscale", [P, NBANK], _DT.float32, kind="ExternalInput").ap()
    wbias_d = nc.dram_tensor("wbias", [P, NBANK], _DT.float32, kind="ExternalInput").ap()
    out_d = nc.dram_tensor("out", [16, HALVES * 512], _DT.int32, kind="ExternalOutput").ap()

    with tile.TileContext(nc) as tc:
        with (
            tc.tile_pool(name="const", bufs=1) as cpool,
            tc.tile_pool(name="x", bufs=2) as xpool,
            tc.tile_pool(name="oh", bufs=6) as opool,
            tc.tile_pool(name="psum", bufs=2, space="PSUM") as ppool,
            tc.tile_pool(name="enc", bufs=4) as epool,
            tc.tile_pool(name="small", bufs=4) as spool,
            tc.tile_pool(name="sqp", bufs=2) as sqpool,
            tc.tile_pool(name="outp", bufs=1) as outpool,
        ):
            # consts ride the SWDGE queue so the x DMAs own the sync queue
            lhstb = cpool.tile([P, 12 * P], _DT.bfloat16)
            nc.gpsimd.dma_start(out=lhstb[:], in_=lhstb_d)
            lhst8 = cpool.tile([P, 2 * 12 * P], _DT.float8e4)
            nc.gpsimd.dma_start(out=lhst8[:], in_=lhst8_d)
            actb = cpool.tile([P, 2 * len(ACT_CLS)], _DT.float32)
            nc.gpsimd.dma_start(out=actb[:], in_=actb_d)
            wscale = cpool.tile([P, NBANK], _DT.float32)
            nc.gpsimd.dma_start(out=wscale[:], in_=wscale_d)
            wbias = cpool.tile([P, NBANK], _DT.float32)
            nc.gpsimd.dma_start(out=wbias[:], in_=wbias_d)

            out_t = outpool.tile([16, HALVES * 512], _DT.int32)

            for hf in range(HALVES):
                xt = xpool.tile([P, HFREE], _DT.bfloat16)
                nc.sync.dma_start(out=xt[:], in_=x_d[:, hf * HFREE:(hf + 1) * HFREE])

                banks = [
                    ppool.tile([P, 512], _DT.float32, name=f"bank{g}", tag=f"bank{g}")
                    for g in range(NBANK)
                ]
                # per bank, emit DVE classes first so chains start promptly
                bank_cls = [[], [], []]
                for c in DVE_BF16 + DVE_FP8 + ACT_CLS + GP_CLS:
                    bank_cls[c // 8].append(c)
                for g in range(NBANK):
                    for pos, c in enumerate(bank_cls[g]):
                        j = c % 8
                        first = pos == 0
                        last = pos == len(bank_cls[g]) - 1
                        use_fp8 = c not in DVE_BF16
                        if c in DVE_BF16 or c in DVE_FP8:
                            oh = opool.tile(
                                [P, HFREE],
                                _DT.float8e4 if use_fp8 else _DT.bfloat16,
                                name=f"oh{'8' if use_fp8 else 'b'}",
                                tag=f"oh{'8' if use_fp8 else 'b'}",
                            )
                            nc.vector.tensor_scalar(
                                out=oh[:], in0=xt[:],
                                scalar1=float(c), scalar2=None,
                                op0=mybir.AluOpType.is_equal,
                            )
                        elif c in GP_CLS:
                            oh = opool.tile([P, HFREE], _DT.float8e4,
                                            name="oh8", tag="oh8")
                            nc.gpsimd.tensor_scalar(
                                out=oh[:], in0=xt[:],
                                scalar1=float(c), scalar2=None,
                                op0=mybir.AluOpType.is_equal,
                            )
                        else:  # ACT: relu(1 - (x - c)^2), exact 0/1 after cast
                            ai = ACT_CLS.index(c)
                            sq = sqpool.tile([P, HFREE], _DT.float32,
                                             name="sq", tag="sq")
                            nc.scalar.activation(
                                sq[:], xt[:], mybir.ActivationFunctionType.Square,
                                bias=actb[:, 2 * ai:2 * ai + 1], scale=1.0,
                            )
                            oh = opool.tile([P, HFREE], _DT.float8e4,
                                            name="oh8", tag="oh8")
                            nc.scalar.activation(
                                oh[:], sq[:], mybir.ActivationFunctionType.Relu,
                                bias=actb[:, 2 * ai + 1:2 * ai + 2], scale=-1.0,
                            )
                        # lhsT block: doubled variants for classes 16..19
                        lj = (8 + j) if g == 2 else j
                        if use_fp8:
                            lt = lhst8[:, 2 * lj * P:2 * (lj + 1) * P].rearrange(
                                "p (t m) -> p t m", t=2)
                            oh3 = oh[:].rearrange(
                                "p (rp t n) -> p rp t n",
                                rp=DSF // 2, t=2, n=512)
                            for rp in range(DSF // 2):
                                nc.tensor.matmul(
                                    banks[g][:, :],
                                    lt,
                                    oh3[:, rp],
                                    start=(first and rp == 0),
                                    stop=(last and rp == DSF // 2 - 1),
                                    perf_mode=mybir.MatmulPerfMode.DoubleRow,
                                )
                        else:
                            lt = lhstb[:, lj * P:(lj + 1) * P]
                            for r in range(DSF):
                                nc.tensor.matmul(
                                    banks[g][:, :],
                                    lt,
                                    oh[:, r * 512:(r + 1) * 512],
                                    start=(first and r == 0),
                                    stop=(last and r == DSF - 1),
                                )

                # tail ops jump the engine queues as soon as deps allow
                hp = tc.high_priority()
                hp.__enter__()
                # E = count * (64*w_c) - c   (ACT: per-partition APs)
                encs = []
                for g in range(NBANK):
                    e = epool.tile([P, 512], _DT.float32)
                    nc.scalar.activation(
                        e[:], banks[g][:], mybir.ActivationFunctionType.Identity,
                        bias=wbias[:, g:g + 1], scale=wscale[:, g:g + 1],
                    )
                    encs.append(e)

                # max across banks (same partition layout)
                m01 = epool.tile([P, 512], _DT.float32)
                nc.vector.tensor_tensor(
                    out=m01[:], in0=encs[0][:], in1=encs[1][:],
                    op=mybir.AluOpType.max,
                )
                m = epool.tile([P, 512], _DT.float32)
                nc.vector.tensor_tensor(
                    out=m[:], in0=m01[:], in1=encs[2][:],
                    op=mybir.AluOpType.max,
                )

                # fold partition groups 128 -> 16 (move with DMA, then max)
                cur = m
                for width in (64, 32, 16):
                    moved = spool.tile([width, 512], _DT.float32,
                                       name="moved", tag="moved", bufs=2)
                    nc.sync.dma_start(out=moved[:], in_=cur[width:2 * width, :])
                    nxt = spool.tile([width, 512], _DT.float32,
                                     name="nxt", tag="nxt", bufs=2)
                    nc.vector.tensor_tensor(
                        out=nxt[:], in0=cur[:width, :], in1=moved[:],
                        op=mybir.AluOpType.max,
                    )
                    cur = nxt

                # decode: F = 64W - c; W = cast((F + 25) / 64) (frac =
                # (25 - c)/64 in (0, 0.5): trunc and round-nearest agree);
                # c* = 64W - F, with casts fused into the tensor ops.
                f = cur  # [16, 512] fp32
                wi = spool.tile([16, 512], _DT.int32, name="wi", tag="wi", bufs=2)
                nc.vector.tensor_scalar(
                    out=wi[:], in0=f[:],
                    scalar1=25.0, scalar2=1.0 / 64.0,
                    op0=mybir.AluOpType.add, op1=mybir.AluOpType.mult,
                )
                nc.vector.scalar_tensor_tensor(
                    out=out_t[:, hf * 512:(hf + 1) * 512], in0=wi[:],
                    scalar=64.0, in1=f[:],
                    op0=mybir.AluOpType.mult, op1=mybir.AluOpType.subtract,
                )
                nc.sync.dma_start(
                    out=out_d[:, hf * 512:(hf + 1) * 512],
                    in_=out_t[:, hf * 512:(hf + 1) * 512],
                )
                hp.__exit__(None, None, None)
    nc.finalize()
    return nc


_CACHED = None


def _get_nc():
    global _CACHED
    if _CACHED is None:
        _CACHED = _build()
    return _CACHED


def kernel(labels: np.ndarray, class_weights: np.ndarray, dsf) -> np.ndarray:
    global LAST_RESULTS
    dsf = int(np.asarray(dsf))
    assert dsf == DSF, f"kernel hardcodes dsf=8, got {dsf}"
    labels = np.asarray(labels)
    out_dtype = labels.dtype
    cw = np.asarray(class_weights, dtype=np.float32)

    # host prep: shard rows, extract low 16 bits, transpose to [p, wc*512+row]
    lab = labels.reshape(B * H, W).astype(np.uint16)
    lhst_bf, lhst_f8, actb, wscale, wbias = _aux_arrays(cw)
    in_maps = []
    for k in range(NCORES):
        shard = lab[k * ROWS:(k + 1) * ROWS]                  # [512, 2048]
        # [prow, r, hf, wcl, p] -> [p, hf, r, wcl, prow]
        x = shard.reshape(PROWS, DSF, HALVES, WCH, P).transpose(4, 2, 1, 3, 0)
        x = np.ascontiguousarray(x).astype(ml_dtypes.bfloat16).reshape(P, FREE)
        in_maps.append({
            "x": x,
            "lhstb": lhst_bf,
            "lhst8": lhst_f8,
            "actb": actb,
            "wscale": wscale,
            "wbias": wbias,
        })

    nc = _get_nc()
    res = run_bass_kernel_spmd(
        nc, in_maps, core_ids=list(range(NCORES)), trace=TRACE,
    )
    LAST_RESULTS = res

    # unshard: core k out [16, 1024] int32; out[jj, hf*512 + wcl*64 + prow]
    # -> modes[patch_row = 64k + prow, j = (hf*8 + wcl)*16 + jj]
    modes = np.empty((B * GH, GW), dtype=np.int64)
    for k in range(NCORES):
        o = res.results[k]["out"].reshape(16, HALVES, WCH, PROWS)
        # axes: (jj, hf, wcl, prow) -> [prow, hf, wcl, jj]
        blk = o.transpose(3, 1, 2, 0).reshape(PROWS, WC * 16)
        modes[k * PROWS:(k + 1) * PROWS] = blk
    return modes.reshape(B, GH, GW).astype(out_dtype)



# revision 6
# speedup vs baseline: 1.4405x; 1.4405x over previous
"""ClassWeightedModalDownSampler Trainium2 kernel (packed exponent planes).

Problem: labels [4, 1024, 2048] int (values 0..19), class_weights [20] f32,
dsf=8.  Output modes [4, 128, 256]: per non-overlapping 8x8 patch, the
argmax over classes of (class histogram * class_weights), first-index
tie-break (jnp.argmax semantics).

Key idea: instead of 20 one-hot planes, build FIVE "packed exponent"
planes.  The host uploads u16 = (6*x + 127) << 7 (int16) = the bf16 bit
pattern of 2^(6x) = 64^x.  For plane a (classes 4a..4a+3), ONE fused DVE
op  (u16 min M_a) - 3072*a  clamps the exponent at class 4a+3 and
rebases, so the int16 bits, reinterpreted as bf16, equal

    64^(x-4a)        for x in {4a..4a+2}   (1, 64, 4096)
    64^3 = 262144    for x >= 4a+3         (clamped)
    2^(6(x-4a)) < 1  for x < 4a            (harmless dust, < 0.24/patch)

A ones-lhsT matmul sums each 8x8 patch: the fp32 PSUM value is the exact
4-digit base-64 number  n0 + 64*n1 + 4096*n2 + 262144*U3  where n_j are
class counts and U3 = #{x >= 4a+3} (a cumulative count).  Since
n0+n1+n2+U3 = 64, the sum is <= 2^24 and exact in fp32.  Digit peeling
(ACT truncs with round-nearest-safe biases + DVE remainder STTs)
recovers n/U; chain recovery n_{4a+3} = U3(a) - sum(digits(a+1)) runs as
a small bf16 matmul that directly emits E = 64*w_c*n_c - c for those
classes; E for the direct classes is a per-partition-scalar op.  Max
over classes + the baseline's (F+25)/64 decode give the argmax with
first-index tie-break.

Data parallel over 8 cores (64 patch rows each); per half (8 of 16
column chunks) the layout matches the old kernel: partition p = w mod
128 (16 patch-cols x 8 pixels), free n = r*512 + wcl*64 + prow.
"""

import numpy as np
import ml_dtypes

import concourse.bass as bass
import concourse.mybir as mybir
import concourse.tile as tile
from concourse import bacc
from concourse.bass_utils import run_bass_kernel_spmd

NCORES = 8
B, H, W = 4, 1024, 2048
DSF = 8
NCLS = 20
GH, GW = H // DSF, W // DSF  # 128, 256 output grid
ROWS = (B * H) // NCORES     # 512 label rows per core
PROWS = ROWS // DSF          # 64 patch rows per core
P = 128
WC = W // P                  # 16 column chunks of 128
HALVES = 2
WCH = WC // HALVES           # 8 chunks per half
FREE = WC * ROWS             # 8192
HFREE = FREE // HALVES       # 4096
NPL = 5                      # packed planes (4 classes each)

_DT = mybir.dt
_A = mybir.AluOpType
_AF = mybir.ActivationFunctionType

# Set by test.py to request a traced run.
TRACE = False
LAST_RESULTS = None


def _aux_arrays(class_weights: np.ndarray):
    """Host-built constants: stage-A/recovery lhsT, scalar APs, biases."""
    w = np.asarray(class_weights, dtype=np.float32)

    # stage-A lhsT: plane a sums 8-partition groups into M-slot a*16 + j.
    lhA = np.zeros((P, NPL * P), dtype=np.float32)
    for a in range(NPL):
        for p in range(P):
            lhA[p, a * P + a * 16 + p // 8] = 1.0
    lhA = lhA.astype(ml_dtypes.bfloat16)

    # recovery lhsT: E3[m = a*16+j] = 64*w[4a+3]*(d3(a) - sum_k d_k(a+1)) -
    # (4a+3), one [128,128] block per digit tile k (0..3), bias via
    # partition 120 of the d0 tile (held at 1.0).
    lhR = np.zeros((P, 4 * P), dtype=np.float32)
    for a in range(NPL):
        c = 4 * a + 3
        for j in range(16):
            m = a * 16 + j
            lhR[a * 16 + j, 3 * P + m] = 64.0 * w[c]
            if a + 1 < NPL:
                for k in range(4):
                    lhR[(a + 1) * 16 + j, k * P + m] = -64.0 * w[c]
    for a in range(NPL):
        for j in range(16):
            lhR[96, 0 * P + a * 16 + j] = -(4 * a + 3)
    lhR = lhR.astype(ml_dtypes.bfloat16)

    # per-partition scalars for direct classes: E_jd = 64*w[4a+jd]*d - c
    wap = np.zeros((P, 3), dtype=np.float32)
    cap = np.zeros((P, 3), dtype=np.float32)
    for jd in range(3):
        for m in range(NPL * 16):
            a = m // 16
            wap[m, jd] = 64.0 * w[4 * a + jd]
            cap[m, jd] = float(4 * a + jd)

    # ACT biases: [zero, -0.492 (d3/d2), -0.498 (d1), -0.375 (d0)]
    actb = np.zeros((P, 4), dtype=np.float32)
    actb[:, 1] = -0.492
    actb[:, 2] = -0.498
    actb[:, 3] = -0.375
    return lhA, lhR, wap, cap, actb


def _build():
    nc = bacc.Bacc(
        "TRN2",
        target_bir_lowering=False,
        debug=False,
        num_devices=NCORES,
    )
    u_d = nc.dram_tensor("u", [P, FREE], _DT.int16, kind="ExternalInput").ap()
    lha_d = nc.dram_tensor("lha", [P, NPL * P], _DT.bfloat16, kind="ExternalInput").ap()
    lhr_d = nc.dram_tensor("lhr", [P, 4 * P], _DT.bfloat16, kind="ExternalInput").ap()
    wap_d = nc.dram_tensor("wap", [P, 3], _DT.float32, kind="ExternalInput").ap()
    cap_d = nc.dram_tensor("cap", [P, 3], _DT.float32, kind="ExternalInput").ap()
    actb_d = nc.dram_tensor("actb", [P, 4], _DT.float32, kind="ExternalInput").ap()
    out_d = nc.dram_tensor("out", [16, HALVES * 512], _DT.int32, kind="ExternalOutput").ap()

    with tile.TileContext(nc) as tc:
        with (
            tc.tile_pool(name="const", bufs=1) as cpool,
            tc.tile_pool(name="u", bufs=2) as upool,
            tc.tile_pool(name="pk", bufs=2) as kpool,
            tc.tile_pool(name="psA", bufs=2, space="PSUM") as pApool,
            tc.tile_pool(name="psE", bufs=2, space="PSUM") as pEpool,
            tc.tile_pool(name="tail", bufs=2) as tpool,
            tc.tile_pool(name="outp", bufs=1) as outpool,
        ):
            # consts on the SWDGE queue so the u DMAs own the sync queue
            lhA = cpool.tile([P, NPL * P], _DT.bfloat16)
            nc.gpsimd.dma_start(out=lhA[:], in_=lha_d)
            lhR = cpool.tile([P, 4 * P], _DT.bfloat16)
            nc.gpsimd.dma_start(out=lhR[:], in_=lhr_d)
            wap = cpool.tile([P, 3], _DT.float32)
            nc.gpsimd.dma_start(out=wap[:], in_=wap_d)
            cap = cpool.tile([P, 3], _DT.float32)
            nc.gpsimd.dma_start(out=cap[:], in_=cap_d)
            actb = cpool.tile([P, 4], _DT.float32)
            nc.gpsimd.dma_start(out=actb[:], in_=actb_d)

            out_t = outpool.tile([16, HALVES * 512], _DT.int32)

            for hf in range(HALVES):
                ut = upool.tile([P, HFREE], _DT.int16)
                nc.sync.dma_start(out=ut[:], in_=u_d[:, hf * HFREE:(hf + 1) * HFREE])

                # stage A: 5 packed planes -> one PSUM bank
                bank = pApool.tile([P, 512], _DT.float32, name="S", tag="S")
                for a in range(NPL):
                    pk = kpool.tile([P, HFREE], _DT.int16, name=f"pk{a}", tag=f"pk{a}")
                    nc.vector.tensor_scalar(
                        out=pk[:], in0=ut[:],
                        scalar1=float((145 + 24 * a) * 128),
                        scalar2=float(3072 * a),
                        op0=_A.min, op1=_A.subtract,
                    )
                    rhs = pk[:].bitcast(_DT.bfloat16)
                    for r in range(DSF):
                        nc.tensor.matmul(
                            bank[:, :],
                            lhA[:, a * P:(a + 1) * P],
                            rhs[:, r * 512:(r + 1) * 512],
                            start=(a == 0 and r == 0),
                            stop=(a == NPL - 1 and r == DSF - 1),
                        )

                hp = tc.high_priority()
                hp.__enter__()

                # digit peel: S -> d3,d2,d1,d0 (int16) + remainders
                s80 = bank[0:80, :]
                S = tpool.tile([80, 512], _DT.float32, name="Ssb", tag="Ssb")
                nc.scalar.activation(S[:], s80, _AF.Identity,
                                     bias=actb[0:80, 0:1], scale=1.0)
                d3i = tpool.tile([80, 512], _DT.int16, name="d3i", tag="d3i")
                nc.scalar.activation(d3i[:], S[:], _AF.Identity,
                                     bias=actb[0:80, 1:2], scale=1.0 / 262144)
                rem2 = tpool.tile([80, 512], _DT.float32, name="rem2", tag="rem2")
                nc.vector.scalar_tensor_tensor(
                    out=rem2[:], in0=d3i[:], scalar=-262144.0, in1=S[:],
                    op0=_A.mult, op1=_A.add)
                d2i = tpool.tile([80, 512], _DT.int16, name="d2i", tag="d2i")
                nc.scalar.activation(d2i[:], rem2[:], _AF.Identity,
                                     bias=actb[0:80, 1:2], scale=1.0 / 4096)
                rem1 = tpool.tile([80, 512], _DT.float32, name="rem1", tag="rem1")
                nc.vector.scalar_tensor_tensor(
                    out=rem1[:], in0=d2i[:], scalar=-4096.0, in1=rem2[:],
                    op0=_A.mult, op1=_A.add)
                d1i = tpool.tile([80, 512], _DT.int16, name="d1i", tag="d1i")
                nc.scalar.activation(d1i[:], rem1[:], _AF.Identity,
                                     bias=actb[0:80, 2:3], scale=1.0 / 64)
                d0f = tpool.tile([80, 512], _DT.float32, name="d0f", tag="d0f")
                nc.vector.scalar_tensor_tensor(
                    out=d0f[:], in0=d1i[:], scalar=-64.0, in1=rem1[:],
                    op0=_A.mult, op1=_A.add)
                d0i = tpool.tile([80, 512], _DT.int16, name="d0i", tag="d0i")
                nc.scalar.activation(d0i[:], d0f[:], _AF.Identity,
                                     bias=actb[0:80, 3:4], scale=1.0)

                # bf16 digit tiles for the recovery matmul (partition 120
                # of the d0 tile carries the -c bias via lhR)
                dbs = []
                for nm, di in (("d0b", d0i), ("d1b", d1i), ("d2b", d2i), ("d3b", d3i)):
                    db = tpool.tile([P, 512], _DT.bfloat16, name=nm, tag=nm)
                    nc.vector.memset(db[:, :], 0.0)
                    nc.vector.tensor_scalar(
                        out=db[0:80, :], in0=di[:], scalar1=0.0, scalar2=None,
                        op0=_A.add)
                    dbs.append(db)
                # partitions 96:128 of the d0 tile hold 1.0; lhR row 96
                # carries the -c bias (rows 97+ have zero coefficients)
                nc.vector.memset(dbs[0][96:128, :], 1.0)

                # E for chain classes (4a+3): small recovery matmul
                ps2 = pEpool.tile([P, 512], _DT.float32, name="E3", tag="E3")
                for k in range(4):
                    nc.tensor.matmul(
                        ps2[:, :], lhR[:, k * P:(k + 1) * P], dbs[k][:, :],
                        start=(k == 0), stop=(k == 3),
                    )

                # E for direct classes: per-partition scalars on GPSIMD
                es = []
                for jd, di in ((0, d0i), (1, d1i), (2, d2i)):
                    e = tpool.tile([80, 512], _DT.float32, name=f"e{jd}", tag=f"e{jd}")
                    nc.gpsimd.tensor_scalar(
                        out=e[:], in0=di[:],
                        scalar1=wap[0:80, jd:jd + 1], scalar2=cap[0:80, jd:jd + 1],
                        op0=_A.mult, op1=_A.subtract,
                    )
                    es.append(e)

                # max over the 4 E sources
                m01 = tpool.tile([80, 512], _DT.float32, name="m01", tag="m01")
                nc.vector.tensor_tensor(out=m01[:], in0=es[0][:], in1=es[1][:],
                                        op=_A.max)
                m2 = tpool.tile([80, 512], _DT.float32, name="m2", tag="m2")
                nc.vector.tensor_tensor(out=m2[:], in0=m01[:], in1=es[2][:],
                                        op=_A.max)
                m3 = tpool.tile([80, 512], _DT.float32, name="m3", tag="m3")
                nc.vector.tensor_tensor(out=m3[:], in0=m2[:], in1=ps2[0:80, :],
                                        op=_A.max)

                # fold the 5 plane-partitions (a*16+j) down to 16 (j)
                t1 = tpool.tile([32, 512], _DT.float32, name="t1", tag="t1")
                nc.sync.dma_start(out=t1[:], in_=m3[32:64, :])
                f1 = tpool.tile([32, 512], _DT.float32, name="f1", tag="f1")
                nc.vector.tensor_tensor(out=f1[:], in0=m3[0:32, :], in1=t1[:],
                                        op=_A.max)
                t2 = tpool.tile([16, 512], _DT.float32, name="t2", tag="t2")
                nc.sync.dma_start(out=t2[:], in_=f1[16:32, :])
                f2 = tpool.tile([16, 512], _DT.float32, name="f2", tag="f2")
                nc.vector.tensor_tensor(out=f2[:], in0=f1[0:16, :], in1=t2[:],
                                        op=_A.max)
                t3 = tpool.tile([16, 512], _DT.float32, name="t3", tag="t3")
                nc.sync.dma_start(out=t3[:], in_=m3[64:80, :])
                f3 = tpool.tile([16, 512], _DT.float32, name="f3", tag="f3")
                nc.vector.tensor_tensor(out=f3[:], in0=f2[:], in1=t3[:],
                                        op=_A.max)

                # decode: F = 64*w*n - c; W = cast((F + 25)/64); c* = 64W - F
                wi = tpool.tile([16, 512], _DT.int32, name="wi", tag="wi")
                nc.vector.tensor_scalar(
                    out=wi[:], in0=f3[:],
                    scalar1=25.0, scalar2=1.0 / 64.0,
                    op0=_A.add, op1=_A.mult,
                )
                nc.vector.scalar_tensor_tensor(
                    out=out_t[:, hf * 512:(hf + 1) * 512], in0=wi[:],
                    scalar=64.0, in1=f3[:],
                    op0=_A.mult, op1=_A.subtract,
                )
                nc.sync.dma_start(
                    out=out_d[:, hf * 512:(hf + 1) * 512],
                    in_=out_t[:, hf * 512:(hf + 1) * 512],
                )
                hp.__exit__(None, None, None)
    nc.finalize()
    return nc


_CACHED = None


def _get_nc():
    global _CACHED
    if _CACHED is None:
        _CACHED = _build()
    return _CACHED


def kernel(labels: np.ndarray, class_weights: np.ndarray, dsf) -> np.ndarray:
    global LAST_RESULTS
    dsf = int(np.asarray(dsf))
    assert dsf == DSF, f"kernel hardcodes dsf=8, got {dsf}"
    labels = np.asarray(labels)
    out_dtype = labels.dtype
    cw = np.asarray(class_weights, dtype=np.float32)

    # host prep: shard rows, encode u16 = (6x+127)<<7, transpose to
    # [p, hf, r, wcl, prow] (identical layout to the bf16 baseline)
    lab = labels.reshape(B * H, W).astype(np.int16)
    u_all = ((6 * lab + 127) << 7).astype(np.int16)
    lhA, lhR, wap, cap, actb = _aux_arrays(cw)
    in_maps = []
    for k in range(NCORES):
        shard = u_all[k * ROWS:(k + 1) * ROWS]                # [512, 2048]
        u = shard.reshape(PROWS, DSF, HALVES, WCH, P).transpose(4, 2, 1, 3, 0)
        u = np.ascontiguousarray(u).reshape(P, FREE)
        in_maps.append({
            "u": u,
            "lha": lhA,
            "lhr": lhR,
            "wap": wap,
            "cap": cap,
            "actb": actb,
        })

    nc = _get_nc()
    res = run_bass_kernel_spmd(
        nc, in_maps, core_ids=list(range(NCORES)), trace=TRACE,
    )
    LAST_RESULTS = res

    # unshard: core k out [16, 1024] int32; out[jj, hf*512 + wcl*64 + prow]
    # -> modes[patch_row = 64k + prow, j = (hf*8 + wcl)*16 + jj]
    modes = np.empty((B * GH, GW), dtype=np.int64)
    for k in range(NCORES):
        o = res.results[k]["out"].reshape(16, HALVES, WCH, PROWS)
        blk = o.transpose(3, 1, 2, 0).reshape(PROWS, WC * 16)
        modes[k * PROWS:(k + 1) * PROWS] = blk
    return modes.reshape(B, GH, GW).astype(out_dtype)
